# revision 12
# baseline (speedup 1.0000x reference)
import numpy as np
import jax
import jax.numpy as jnp
from jax.sharding import Mesh, PartitionSpec as P, NamedSharding
try:
    from jax.experimental.shard_map import shard_map
except ImportError:
    from jax.shard_map import shard_map  # newer jax

# GPT-MoD dims (hardcoded per problem spec)
B, T, V, C, H, L = 4, 1024, 50257, 768, 6, 6
HS = C // H                 # 128
FF = 4 * C                  # 3072
NEG = -1e30
NB, NT = 4, 2               # mesh: 4-way batch parallel x 2-way tensor parallel
ND = NB * NT
VP = ((V + ND - 1) // ND) * ND   # 50264
VS = VP // ND                    # 6283
HL = H // NT                     # 3 heads per TP rank
CL = HL * HS                     # 384
FL = FF // NT                    # 1536

_mesh = None


def _get_mesh():
    global _mesh
    if _mesh is None:
        devs = np.array(jax.devices()[:ND]).reshape(NB, NT)
        _mesh = Mesh(devs, ('b', 't'))
    return _mesh


def _ln(x, g, b):
    m = x.mean(-1, keepdims=True)
    v = x.var(-1, keepdims=True)
    return (x - m) * jax.lax.rsqrt(v + 1e-5) * g + b


def _fwd_local(x0, ra_w, ra_b, l1g, l1b, l2g, l2b,
               wqkv, pw, pb, f1w, f1b, f2w, f2b,
               lnfg, lnfb, lmw):
    # Body runs entirely in f32: MoD routing thresholds (x @ aux_w > 0) are
    # discontinuous, so the residual stream must track the f32 reference
    # bit-closely or token selections flip and produce O(1) logit errors.
    # Only the lm_head (feeds nothing downstream) runs in bf16.
    # x0: [1,T,C] f32 (local batch shard, replicated over 't')
    # ra_w: [L,C,2] f32 (router|aux)  ra_b: [L,2] f32
    # wqkv: [L,C,3*CL] f32 (local cols = [q 3heads | k 3heads | v 3heads])
    # pw: [L,CL,C] f32  f1w: [L,C,FL] f32  f1b: [L,FL] f32  f2w: [L,FL,C] f32
    # lmw: [C+1,VS] f16 (last row = lm_b)
    f32 = jnp.float32
    bf16 = jnp.bfloat16
    tril_bias = jnp.where(jnp.tril(jnp.ones((T, T), bool)), 0.0, NEG).astype(f32)
    scale = HS ** -0.5
    x = x0
    for l in range(L):
        ra = jnp.einsum('tc,cd->td', x[0], ra_w[l]) + ra_b[l]    # [T,2]
        rw = ra[:, 0][None]                                      # [1,T]
        sel = ra[:, 1][None] > 0.0                               # [1,T]
        h = _ln(x, l1g[l], l1b[l])[0]                            # [T,C]
        qkv = jnp.einsum('tc,cd->td', h, wqkv[l])                # [T,3*CL]
        q = qkv[:, :CL].reshape(T, HL, HS).transpose(1, 0, 2)    # [HL,T,HS]
        k = qkv[:, CL:2 * CL].reshape(T, HL, HS).transpose(1, 0, 2)
        v = qkv[:, 2 * CL:].reshape(T, HL, HS).transpose(1, 0, 2)
        scores = jnp.einsum('htd,hsd->hts', q, k) * scale
        key_bias = jnp.where(sel[0], 0.0, NEG).astype(f32)       # [T]
        # masked entries are -1e30 -> exp underflows to exactly 0; max-sub
        # skipped (scores bounded). Fully-masked query rows give 0/0 = NaN,
        # but those rows are always unselected and discarded by the final
        # jnp.where (a selected query always has its own diagonal key).
        e = jnp.exp(scores + tril_bias[None] + key_bias[None, None, :])
        # divide after the att matmul: normalizes [HL,T,HS] instead of
        # materializing wei over [HL,T,T] (saves a full pass per layer)
        att = jnp.einsum('hts,hsd->htd', e, v) / e.sum(-1, keepdims=True)
        attf = att.transpose(1, 0, 2).reshape(T, CL)             # [T,CL]
        y_part = jnp.einsum('tc,cd->td', attf, pw[l])
        y = x + (jax.lax.psum(y_part, 't') + pb[l])[None]
        h2 = _ln(y, l2g[l], l2b[l])[0]
        a1 = jax.nn.relu(jnp.einsum('tc,cf->tf', h2, f1w[l]) + f1b[l])
        f_part = jnp.einsum('tf,fc->tc', a1, f2w[l])
        f = jax.lax.psum(f_part, 't') + f2b[l]
        blk = y + f[None]
        x = jnp.where(sel[..., None], blk * rw[..., None], x)
    f16 = jnp.float16
    xf = _ln(x, lnfg, lnfb).astype(f16)                     # [1,T,C]
    xg = jax.lax.all_gather(xf, 'b', axis=0, tiled=True)    # [B,T,C]
    ones = jnp.ones((B, T, 1), f16)
    xa = jnp.concatenate([xg, ones], axis=-1)               # [B,T,C+1]
    logits = jnp.einsum('btc,cv->btv', xa, lmw,
                        preferred_element_type=f16)
    return logits                                           # [B,T,VS] f16 local


_run_jit = None


def _get_run():
    global _run_jit
    if _run_jit is None:
        mesh = _get_mesh()
        rep = P()
        in_specs = (
            P('b', None, None),            # x0
            rep, rep,                      # ra_w, ra_b
            rep, rep, rep, rep,            # ln1_g/b, ln2_g/b
            P(None, None, 't'),            # wqkv (rank-blocked cols)
            P(None, 't', None),            # pw
            rep,                           # pb
            P(None, None, 't'),            # f1w
            P(None, 't'),                  # f1b
            P(None, 't', None),            # f2w
            rep,                           # f2b
            rep, rep,                      # lnf_g/b
            P(None, ('b', 't')),           # lmw (with bias row)
        )
        out_specs = P(None, None, ('b', 't'))
        _run_jit = jax.jit(shard_map(
            _fwd_local, mesh=mesh, in_specs=in_specs, out_specs=out_specs,
            check_rep=False))
    return _run_jit


def prepare(inputs):
    """Host-side preprocessing + staging onto the 8 cores. Returns arg tuple."""
    inp = {k: np.asarray(v) for k, v in inputs.items()}
    idx = inp['idx'].astype(np.int64)
    # embedding gather on host: avoids shipping the 154MB table over the tunnel
    x0 = inp['tok_emb'][idx].astype(np.float32) + inp['pos_emb'][None].astype(np.float32)

    def flat_qkv(w):  # [L,H,C,HS] -> [L,C,H*HS] with col = h*HS+d
        return np.ascontiguousarray(w.transpose(0, 2, 1, 3).reshape(L, C, H * HS))

    # merged qkv with rank-blocked columns: rank t gets [q heads(3t..) | k | v]
    wq, wk, wv = flat_qkv(inp['wq']), flat_qkv(inp['wk']), flat_qkv(inp['wv'])
    blocks = []
    for t in range(NT):
        sl = slice(t * CL, (t + 1) * CL)
        blocks.append(np.concatenate([wq[:, :, sl], wk[:, :, sl], wv[:, :, sl]], axis=2))
    wqkv = np.ascontiguousarray(np.concatenate(blocks, axis=2))  # [L,C,NT*3CL]

    ra_w = np.stack([inp['router_w'], inp['aux_w']], axis=2).astype(np.float32)  # [L,C,2]
    ra_b = np.stack([inp['router_b'], inp['aux_b']], axis=1).astype(np.float32)  # [L,2]

    lm_w = np.zeros((C + 1, VP), np.float32)
    lm_w[:C, :V] = inp['lm_w']
    lm_w[C, :V] = inp['lm_b']

    host_args = (
        x0,
        ra_w, ra_b,
        inp['ln1_g'].astype(np.float32), inp['ln1_b'].astype(np.float32),
        inp['ln2_g'].astype(np.float32), inp['ln2_b'].astype(np.float32),
        wqkv,
        inp['proj_w'].astype(np.float32), inp['proj_b'].astype(np.float32),
        inp['ffn_w1'].astype(np.float32), inp['ffn_b1'].astype(np.float32),
        inp['ffn_w2'].astype(np.float32), inp['ffn_b2'].astype(np.float32),
        inp['lnf_g'].astype(np.float32), inp['lnf_b'].astype(np.float32),
        lm_w.astype(np.float16),
    )
    mesh = _get_mesh()
    rep = P()
    specs = (
        P('b', None, None),
        rep, rep,
        rep, rep, rep, rep,
        P(None, None, 't'),
        P(None, 't', None), rep,
        P(None, None, 't'), P(None, 't'),
        P(None, 't', None), rep,
        rep, rep,
        P(None, ('b', 't')),
    )
    staged = []
    for a, s in zip(host_args, specs):
        staged.append(jax.device_put(a, NamedSharding(mesh, s)))
    return tuple(staged)


def run(staged):
    return _get_run()(*staged)


def kernel(**inputs):
    try:
        staged = prepare(inputs)
        out = run(staged)                     # [B,T,VP] f16, vocab-sharded
        logits = np.asarray(out).astype(np.float32)[:, :, :V]
        return np.ascontiguousarray(logits)
    except Exception:
        return _kernel_fallback(**inputs)


# ----- single-device fallback (correctness safety net) -----

def _kernel_fallback(**inputs):
    inp = {k: np.asarray(v) for k, v in inputs.items()}
    idx = jnp.asarray(inp['idx'].astype(np.int32))
    f = jax.jit(_fallback_body)
    x = f(idx, *[jnp.asarray(inp[k].astype(np.float32)) for k in
                 ('tok_emb', 'pos_emb', 'router_w', 'router_b', 'aux_w', 'aux_b',
                  'ln1_g', 'ln1_b', 'ln2_g', 'ln2_b', 'wq', 'wk', 'wv',
                  'proj_w', 'proj_b', 'ffn_w1', 'ffn_b1', 'ffn_w2', 'ffn_b2',
                  'lnf_g', 'lnf_b')])
    logits = np.asarray(jnp.asarray(x) @ inp['lm_w'].astype(np.float32)
                        + inp['lm_b'].astype(np.float32))
    return np.ascontiguousarray(logits)


def _fallback_body(idx, tok_emb, pos_emb, router_w, router_b, aux_w, aux_b,
                   ln1_g, ln1_b, ln2_g, ln2_b, wq, wk, wv, proj_w, proj_b,
                   ffn_w1, ffn_b1, ffn_w2, ffn_b2, lnf_g, lnf_b):
    x = tok_emb[idx] + pos_emb[None, :, :]
    tril = jnp.tril(jnp.ones((T, T), bool))

    def layer(x, w):
        (rw_w, rw_b, aw, ab, l1g, l1b, l2g, l2b,
         wq_l, wk_l, wv_l, pw, pb, f1w, f1b, f2w, f2b) = w
        rw = x @ rw_w + rw_b
        sel = (x @ aw + ab) > 0.0
        h = _ln(x, l1g, l1b)
        q = jnp.einsum('btc,hcd->bhtd', h, wq_l)
        k = jnp.einsum('btc,hcd->bhtd', h, wk_l)
        v = jnp.einsum('btc,hcd->bhtd', h, wv_l)
        scores = jnp.einsum('bhtd,bhsd->bhts', q, k) * (HS ** -0.5)
        mask = sel[:, None, :, None] & sel[:, None, None, :] & tril
        wei = jax.nn.softmax(jnp.where(mask, scores, NEG), axis=-1)
        att = jnp.einsum('bhts,bhsd->bhtd', wei, v)
        att = att.transpose(0, 2, 1, 3).reshape(B, T, C)
        y = x + att @ pw + pb
        f = jax.nn.relu(_ln(y, l2g, l2b) @ f1w + f1b) @ f2w + f2b
        blk = y + f
        return jnp.where(sel[..., None], blk * rw[..., None], x), None

    ws = (router_w, router_b, aux_w, aux_b, ln1_g, ln1_b, ln2_g, ln2_b,
          wq, wk, wv, proj_w, proj_b, ffn_w1, ffn_b1, ffn_w2, ffn_b2)
    x, _ = jax.lax.scan(layer, x, ws)
    return _ln(x, lnf_g, lnf_b)
